# revision 2
# baseline (speedup 1.0000x reference)
"""DeformableConv2d Trainium2 kernel, v2.

Data-parallel over batch: 8 samples -> 8 NeuronCores, one sample per core.

Identity: bilinear sampling commutes with the channel contraction, so
G_k = W_k @ x (1x1 conv per tap) is computed first and the bilinear gather
fetches G directly; the gathered values only need a per-pixel weighted
4-sum (no post-gather matmul).

Key improvements over v1:
  * Gather grid stores row-PAIR cells in two parity copies (A: rows
    (2y,2y+1), B: rows (2y+1,2y+2)).  One dma_gather element of 512B
    (elem_size=256, elem_step=128) fetches the full 2x2 bilinear quad
    [G(y0,x0),G(y1,x0),G(y0,x1),G(y1,x1)] regardless of y0 parity ->
    one gather per (tap, half) at full DMA bandwidth (512B descriptors
    avoid the <512B 2x DMA latency penalty).
  * Combine uses premultiplied bilinear weights (4 mul + 4 add per quad)
    with weights duplicated pairwise in memory so every DVE op has a
    packed bf16 last dim -> 2x DVE mode throughout.
  * Offset conv folds tap pairs (ki,ki+1) into K=128 matmuls using a
    row-shifted copy of x on partitions 64..127.
  * G conv runs even/odd image-row matmuls so PSUM drains produce
    pair-partitioned staging tiles; the A-grid DMA write is fully
    contiguous (4KB runs) and the B-grid tile is built by two cheap
    DVE copies.
"""

import numpy as np

K = 3
B, CIN, COUT, H, W = 8, 64, 64, 112, 112
HW = H * W                 # 12544
NPAD = H + 2               # 114
NQB = HW // 128            # 98
NPLANE = 2 * K * K         # 18
HALF = HW // 2             # 6272
HSLOT = HALF // 128        # 49
GPAD = 4
GP = H + 2 * GPAD          # 120 grid columns
# Pair-row grid layout per tap (rows of GP cells each):
#   rows 0..59   : A-parity pairs (2y, 2y+1), data in rows 2..57
#   rows 66..122 : B-parity pairs (2y+1, 2y+2) at row 65+yp', data rows
#                  66..122; rows 58..65 and 123 stay zero
# B at a 64-aligned partition offset lets one 121-partition DMA write
# both grids (partition p -> grid row p+2).
BROW0 = 65                 # B pair yp' -> grid row BROW0 + yp'
POFF = BROW0 * GP          # cell-index offset for parity 1 (7800)
NROWS = 124                # grid rows per tap
TAPC = NROWS * GP          # 14880 cells per tap
XG = 16                    # image columns per staging group
NXG = W // XG              # 7
M23 = 8388608.0

# offset-conv rounds: tap pairs (k, k+3) share kj and consecutive ki, so
# partitions 64..127 (x shifted one row down) provide tap k+3's window.
OC_ROUNDS = [(0, 3), (1, 4), (2, 5), (6,), (7,), (8,)]


def _build_program(weights, debug_skip=()):
    import ml_dtypes
    import concourse.bass as bass
    import concourse.bacc as bacc
    import concourse.mybir as mybir
    from concourse.tile import TileContext

    dt = mybir.dt
    ALU = mybir.AluOpType
    AP = bass.AP
    bf = ml_dtypes.bfloat16

    nc = bacc.Bacc("TRN2", target_bir_lowering=False, debug=False)

    offset_w = weights["offset_w"]  # [18, 64, 3, 3]
    offset_b = weights["offset_b"]  # [18]
    deform_w = weights["deform_w"]  # [64, 64, 3, 3]
    deform_b = weights["deform_b"]  # [64]

    # ---- host-side constants ----
    # offset conv lhsT per round: [128, 18]; rows 0:64 = first tap of the
    # round, rows 64:128 = second tap (zero for singleton rounds).
    woff = np.zeros((len(OC_ROUNDS), 2 * CIN, NPLANE), np.float32)
    for r, taps in enumerate(OC_ROUNDS):
        for t, k in enumerate(taps):
            ki, kj = k // 3, k % 3
            for j in range(9):
                woff[r, t * CIN:(t + 1) * CIN, j] = offset_w[2 * j, :, ki, kj]
                woff[r, t * CIN:(t + 1) * CIN, 9 + j] = \
                    offset_w[2 * j + 1, :, ki, kj]

    # G conv rhs: wg[c, k*64+o] = deform_w[o, c, ki, kj]
    wg = np.zeros((CIN, 9 * COUT), np.float32)
    for k in range(9):
        ki, kj = k // 3, k % 3
        wg[:, k * COUT:(k + 1) * COUT] = deform_w[:, :, ki, kj].T

    # base sampling grid [128, 18, 98] fp32 (+GPAD baked in, offset_b folded)
    qs = np.arange(HW)
    ho, wo = qs // W, qs % W
    base = np.zeros((NPLANE, HW), np.float32)
    for k in range(9):
        ki, kj = k // 3, k % 3
        base[k] = ho + ki - 1 + GPAD + float(offset_b[2 * k])
        base[9 + k] = wo + kj - 1 + GPAD + float(offset_b[2 * k + 1])
    base_re = np.ascontiguousarray(
        base.reshape(NPLANE, NQB, 128).transpose(2, 0, 1))  # [128, 18, 98]

    bias_rep = np.broadcast_to(
        deform_b.astype(np.float32)[None, :], (128, COUT))

    woff_c = nc.inline_tensor(
        np.ascontiguousarray(woff.transpose(1, 0, 2)).reshape(
            2 * CIN, len(OC_ROUNDS) * NPLANE).astype(bf), name="woffc")
    wg_c = nc.inline_tensor(wg.astype(bf), name="wgc")
    base_c = nc.inline_tensor(
        base_re.reshape(128, NPLANE * NQB), name="basec")
    bias_c = nc.inline_tensor(np.ascontiguousarray(bias_rep).astype(bf),
                              name="biasc")
    ident_c = nc.inline_tensor(np.eye(NPLANE, dtype=np.float32).astype(bf),
                               name="ident")
    # selection matrices for the 16-partition wrap fold (fp32: the folded
    # values are cell indices up to 14399, exact in fp32 matmuls)
    sel = np.zeros((8, 128, 16), np.float32)
    for s in range(8):
        for t in range(16):
            sel[s, 16 * s + t, t] = 1.0
    sel_c = nc.inline_tensor(
        np.ascontiguousarray(sel.transpose(1, 0, 2)).reshape(
            128, 8 * 16), name="selc")

    xpad = nc.dram_tensor("xpad", [2 * CIN, NPAD * NPAD], dt.bfloat16,
                          kind="ExternalInput")
    out_t = nc.dram_tensor("out_t", [HW, COUT], dt.bfloat16,
                           kind="ExternalOutput")
    gws = nc.dram_tensor("gws", [9 * TAPC + 8, 128], dt.bfloat16,
                         kind="Internal")
    gws_ap = gws[:, :]
    gwz = nc.dram_tensor("gwz", [76800], dt.bfloat16, kind="Internal")
    gwz_ap = gwz[:]

    with TileContext(nc) as tc:
        with (
            tc.tile_pool(name="const", bufs=1) as constp,
            tc.tile_pool(name="xsb", bufs=1) as xsbp,
            tc.tile_pool(name="persist", bufs=1) as perp,
            tc.tile_pool(name="accp", bufs=1) as accp,
            tc.tile_pool(name="zz", bufs=1) as zzp,
        ):
            # ---- constants + input ----
            woff_sb = constp.tile([2 * CIN, len(OC_ROUNDS), NPLANE],
                                  dt.bfloat16)
            nc.sync.dma_start(
                out=woff_sb[:],
                in_=woff_c[:].rearrange("c (r m) -> c r m",
                                        r=len(OC_ROUNDS)))
            wg_sb = constp.tile([CIN, 9 * COUT], dt.bfloat16)
            nc.sync.dma_start(out=wg_sb[:], in_=wg_c[:])
            base_sb = constp.tile([128, NPLANE, NQB], dt.float32)
            nc.sync.dma_start(
                out=base_sb[:],
                in_=base_c[:].rearrange("p (a b) -> p a b", a=NPLANE))
            bias_sb = constp.tile([128, COUT], dt.bfloat16)
            nc.sync.dma_start(out=bias_sb[:], in_=bias_c[:])
            ident_sb = constp.tile([NPLANE, NPLANE], dt.bfloat16)
            nc.sync.dma_start(out=ident_sb[:], in_=ident_c[:])
            sel_sb = constp.tile([128, 8, 16], dt.float32)
            nc.sync.dma_start(
                out=sel_sb[:], in_=sel_c[:].rearrange("p (s t) -> p s t", s=8))

            # x2: partitions 0:64 = padded x; 64:128 = same shifted one row
            # down (for the offset-conv tap-pair folding). Built on host.
            x2 = xsbp.tile([128, NPAD, NPAD], dt.bfloat16)
            nc.sync.dma_start(
                out=x2[:],
                in_=xpad[:].rearrange("c (a b) -> c a b", a=NPAD))
            x2ap = x2[:]
            X2S = x2ap.ap[0][0]

            zz = zzp.tile([16, 1920], dt.bfloat16)
            nc.gpsimd.memset(zz[:], 0)
            zzap = zz[:]
            ZS = zzap.ap[0][0]

            # ---- zero-fill of never-written gather-grid cells ----
            # gwz: DRAM zero pool (real-stride source for the cell fills)
            for j in range(5):
                dst = AP(gwz_ap.tensor, j * 15360,
                         [[1920, 8], [1, 1920]])
                src = AP(zzap.tensor, zzap.offset,
                         [[ZS, 8], [1, 1920]])
                nc.sync.dma_start(out=dst, in_=src)
            if "zfill" not in debug_skip:
                for k in range(9):
                    kb = k * TAPC * 128
                    # full zero rows {0,1} and {123} (cell-contiguous runs)
                    for c0, ncell in ((0, 2 * GP), (123 * GP, GP)):
                        dst = AP(gws_ap.tensor, kb + c0 * 128,
                                 [[256, ncell // 2], [1, 256]])
                        src = AP(gwz_ap.tensor, 0,
                                 [[256, ncell // 2], [1, 256]])
                        nc.sync.dma_start(out=dst, in_=src)
                    # x-border cells (4 left + 4 right) for rows 2..122
                    # (covers A data rows, the gap rows, and B data rows)
                    for xoff in (0, GP - GPAD):
                        dst = AP(gws_ap.tensor, kb + (2 * GP + xoff) * 128,
                                 [[GP * 128, 121], [1, 512]])
                        src = AP(gwz_ap.tensor, 0,
                                 [[512, 121], [1, 512]])
                        nc.sync.dma_start(out=dst, in_=src)
                # trailing pad rows (tap 8's gather AP overreads 256 els)
                dst = AP(gws_ap.tensor, 9 * TAPC * 128, [[1, 8 * 128]])
                src = AP(gwz_ap.tensor, 0, [[1, 8 * 128]])
                nc.sync.dma_start(out=dst, in_=src)

            # ---- offset conv + index math (scoped pools) ----
            ab = tc.tile_pool(name="idxm", bufs=1)
            idxp = ab.__enter__()
            psA_cm = tc.tile_pool(name="psA", bufs=2, space="PSUM")
            psA = psA_cm.__enter__()
            psT_cm = tc.tile_pool(name="psT", bufs=2, space="PSUM")
            psT = psT_cm.__enter__()
            psF_cm = tc.tile_pool(name="psF", bufs=2, space="PSUM")
            psF = psF_cm.__enter__()

            off_sb = idxp.tile([NPLANE, HW], dt.bfloat16)
            RB = 4
            for blk in range(H // RB):
                ps = psA.tile([NPLANE, RB * W], dt.float32)
                for r, taps in enumerate(OC_ROUNDS):
                    ki, kj = taps[0] // 3, taps[0] % 3
                    rhs = x2[:, blk * RB + ki: blk * RB + ki + RB,
                             kj: kj + W]
                    nc.tensor.matmul(ps[:], woff_sb[:, r, :], rhs,
                                     start=(r == 0),
                                     stop=(r == len(OC_ROUNDS) - 1))
                nc.scalar.copy(
                    out=off_sb[:, blk * RB * W:(blk + 1) * RB * W], in_=ps[:])

            # [18, HW] -> [128, 18, 98] via PE transposes (4 per drain)
            off_re = idxp.tile([128, NPLANE, NQB], dt.float32)
            for qb0 in range(0, NQB, 4):
                nq = min(4, NQB - qb0)
                pst = psT.tile([128, 4, NPLANE], dt.bfloat16, tag="pst")
                for j in range(nq):
                    qb = qb0 + j
                    nc.tensor.transpose(
                        pst[:, j, :], off_sb[:, qb * 128:(qb + 1) * 128],
                        ident_sb[:])
                nc.scalar.copy(
                    out=off_re[:, :, qb0:qb0 + nq],
                    in_=pst[:, 0:nq, :].rearrange("p a b -> p b a"))

            # ---- index math (DVE, [128, 18, 98] fp32) ----
            pyx = idxp.tile([128, NPLANE, NQB], dt.float32)
            nc.vector.tensor_add(out=pyx[:], in0=off_re[:], in1=base_sb[:])
            rnd = idxp.tile([128, NPLANE, NQB], dt.float32)
            nc.vector.tensor_scalar(rnd[:], pyx[:], M23, M23, ALU.add,
                                    ALU.subtract)
            tmp = idxp.tile([128, NPLANE, NQB], dt.float32)
            nc.vector.tensor_tensor(out=tmp[:], in0=rnd[:], in1=pyx[:],
                                    op=ALU.is_gt)
            flr = rnd
            nc.vector.tensor_sub(out=flr[:], in0=rnd[:], in1=tmp[:])
            frac = pyx
            nc.vector.tensor_sub(out=frac[:], in0=pyx[:], in1=flr[:])

            # premultiplied bilinear weights wq[m] for quad order
            # [ (y0,x0), (y1,x0), (y0,x1), (y1,x1) ]
            nc.vector.tensor_scalar(tmp[:], frac[:], -1.0, 1.0, ALU.mult,
                                    ALU.add)            # tmp = 1 - frac
            wqf = idxp.tile([128, 9, 4, NQB], dt.float32)
            nc.vector.tensor_mul(out=wqf[:, :, 0, :], in0=tmp[:, 0:9],
                                 in1=tmp[:, 9:18])
            nc.vector.tensor_mul(out=wqf[:, :, 1, :], in0=frac[:, 0:9],
                                 in1=tmp[:, 9:18])
            nc.vector.tensor_mul(out=wqf[:, :, 2, :], in0=tmp[:, 0:9],
                                 in1=frac[:, 9:18])
            nc.vector.tensor_mul(out=wqf[:, :, 3, :], in0=frac[:, 0:9],
                                 in1=frac[:, 9:18])
            # duplicate each weight twice (packed last dim -> 2x DVE mode
            # for the combine multiplies)
            wq2 = perp.tile([128, 9, 4, NQB, 2], dt.bfloat16)
            nc.gpsimd.tensor_copy(
                out=wq2[:],
                in_=wqf[:, :, :, :, None].to_broadcast(
                    (128, 9, 4, NQB, 2)))

            # cell index: yh = floor(y0/2), parity = y0 - 2*yh,
            # cell = parity*7200 + yh*120 + x0
            t1 = idxp.tile([128, 9, NQB], dt.float32, tag="t1")
            t2 = idxp.tile([128, 9, NQB], dt.float32, tag="t2")
            y0 = flr[:, 0:9]
            x0 = flr[:, 9:18]
            nc.vector.tensor_scalar_mul(t1[:], y0, 0.5)
            nc.vector.tensor_scalar(t2[:], t1[:], M23, M23, ALU.add,
                                    ALU.subtract)
            nc.vector.tensor_tensor(out=t1[:], in0=t2[:], in1=t1[:],
                                    op=ALU.is_gt)
            nc.vector.tensor_sub(out=t2[:], in0=t2[:], in1=t1[:])  # t2=yh
            nc.vector.scalar_tensor_tensor(
                out=t1[:], in0=t2[:], scalar=-2.0, in1=y0,
                op0=ALU.mult, op1=ALU.add)                       # t1=parity
            nc.vector.scalar_tensor_tensor(
                out=t2[:], in0=t2[:], scalar=float(GP), in1=x0,
                op0=ALU.mult, op1=ALU.add)                       # t2=yh*120+x0
            nc.vector.scalar_tensor_tensor(
                out=t1[:], in0=t1[:], scalar=float(POFF), in1=t2[:],
                op0=ALU.mult, op1=ALU.add)                       # t1=cell
            # +0.25 guards the fp32->int16 truncation in the drain below
            nc.vector.tensor_scalar_add(t1[:], t1[:], 0.25)

            # fold partition q_lo=(16s+t) -> wrapped [16, ...] layout via
            # fp32 selection matmuls (values <= 14399.25, exact); drain
            # converts directly to int16.
            idx_w16 = idxp.tile([16, 9, 2, HSLOT, 8], dt.int16)
            for s in range(8):
                for pc in range(5):   # plane pairs (8 planes) + single
                    npl = 2 if pc < 4 else 1
                    pst2 = psF.tile([16, 2 * NQB], dt.float32, tag="pst2")
                    nc.tensor.matmul(pst2[:, 0:npl * NQB], sel_sb[:, s, :],
                                     t1[:, 2 * pc:2 * pc + npl, :],
                                     start=True, stop=True)
                    nc.scalar.copy(
                        out=idx_w16[:, 2 * pc:2 * pc + npl, :, :, s],
                        in_=pst2[:, 0:npl * NQB].rearrange(
                            "p (a h q) -> p a h q", a=npl, h=2))
            # replicate partitions 0:16 -> all 128 (3 doubling DMAs)
            idx_w = perp.tile([128, 9, 2, 8 * HSLOT], dt.int16)
            i16 = idx_w16[:]
            wap = idx_w[:]
            WSPAN = wap.ap[0][0]
            ISPAN = i16.ap[0][0]
            FREE = 9 * 2 * HSLOT * 8
            nc.sync.dma_start(
                out=AP(wap.tensor, wap.offset, [[WSPAN, 16], [1, FREE]]),
                in_=AP(i16.tensor, i16.offset, [[ISPAN, 16], [1, FREE]]))
            for np_ in (16, 32, 64):
                nc.sync.dma_start(
                    out=AP(wap.tensor, wap.offset + np_ * WSPAN,
                           [[WSPAN, np_], [1, FREE]]),
                    in_=AP(wap.tensor, wap.offset,
                           [[WSPAN, np_], [1, FREE]]))

            psF_cm.__exit__(None, None, None)
            psT_cm.__exit__(None, None, None)
            psA_cm.__exit__(None, None, None)
            ab.__exit__(None, None, None)

            # ---- accumulator init ----
            acc = accp.tile([128, NQB, COUT], dt.bfloat16)
            nc.gpsimd.tensor_copy(
                out=acc[:],
                in_=bias_sb[:, None, :].to_broadcast((128, NQB, COUT)))

            # ---- G staging + gather + combine ----
            # Three row-phase matmul passes per column pair (lhsT selects
            # xpad rows; xpad rows 0/113 are zero padding, giving the
            # B-grid's out-of-image halves for free):
            #   e  [57]: partition j = image row 2j    (xpad 1+2j)
            #   o  [56]: partition j = image row 2j+1  (xpad 2+2j)
            #   o2 [57]: partition j = image row 2j-1  (xpad 2j)
            # gsbA partition p = image rows (2p, 2p+1)   -> A-grid yp 2..57
            # gsbB partition j = image rows (2j-1, 2j)   -> B-grid yp 1..57
            psE_cm = tc.tile_pool(name="psE", bufs=1, space="PSUM")
            psE = psE_cm.__enter__()
            psO_cm = tc.tile_pool(name="psO", bufs=1, space="PSUM")
            psO = psO_cm.__enter__()
            psO2_cm = tc.tile_pool(name="psO2", bufs=1, space="PSUM")
            psO2 = psO2_cm.__enter__()
            gsA_cm = tc.tile_pool(name="gsA", bufs=1)
            gsAp = gsA_cm.__enter__()
            gat_cm = tc.tile_pool(name="gat", bufs=2)
            gatp = gat_cm.__enter__()

            # combined staging tiles (manual double buffer so the one-time
            # gap memset keeps a single tensor identity):
            #   partition p -> grid row p+2
            #   p 0..55   = A pairs (rows 2..57)
            #   p 56..63  = zero gap rows 58..65 (memset once)
            #   p 64..120 = B pairs (rows 66..122)
            gsAB0 = gsAp.tile([121, 5, XG, 2, COUT], dt.bfloat16,
                              tag="gsAB0")
            gsAB1 = gsAp.tile([121, 5, XG, 2, COUT], dt.bfloat16,
                              tag="gsAB1")
            nc.gpsimd.memset(gsAB0[32:64], 0)
            nc.gpsimd.memset(gsAB1[32:64], 0)
            TGROUPS = ((0, 5), (5, 9)) if "gstage" not in debug_skip else ()
            for gi, (kl, kh) in enumerate(TGROUPS):
                NK = kh - kl
                for xg in range(NXG):
                    gsAB = (gsAB0, gsAB1)[xg % 2]
                    for cg in range(XG // 2):
                        cols = (xg * XG + cg * 2, xg * XG + cg * 2 + 1)
                        pe = psE.tile([57, 2, 512], dt.float32, tag="pe")
                        po = psO.tile([56, 2, 512], dt.float32, tag="po")
                        po2 = psO2.tile([57, 2, 512], dt.float32,
                                        tag="po2")
                        for cc, xc in enumerate(cols):
                            for ps, row0, nr in ((pe, 1, 57), (po, 2, 56),
                                                 (po2, 0, 57)):
                                lhsT = AP(x2ap.tensor,
                                          x2ap.offset + row0 * NPAD
                                          + xc + 1,
                                          [[X2S, CIN], [2 * NPAD, nr]])
                                nc.tensor.matmul(
                                    ps[0:nr, cc, 0:NK * COUT], lhsT,
                                    wg_sb[:, kl * COUT:kh * COUT],
                                    start=True, stop=True)
                        c0 = cg * 2
                        # A-h0 <- e[0:56], A-h1 <- o
                        nc.scalar.copy(
                            out=gsAB[0:56, 0:NK, c0:c0 + 2, 0, :],
                            in_=pe[0:56, :, 0:NK * COUT].rearrange(
                                "p c (k o) -> p k c o", k=NK))
                        nc.scalar.copy(
                            out=gsAB[0:56, 0:NK, c0:c0 + 2, 1, :],
                            in_=po[:, :, 0:NK * COUT].rearrange(
                                "p c (k o) -> p k c o", k=NK))
                        # B-h0 <- o2, B-h1 <- e (partition start 64)
                        nc.scalar.copy(
                            out=gsAB[64:121, 0:NK, c0:c0 + 2, 0, :],
                            in_=po2[:, :, 0:NK * COUT].rearrange(
                                "p c (k o) -> p k c o", k=NK))
                        nc.scalar.copy(
                            out=gsAB[64:121, 0:NK, c0:c0 + 2, 1, :],
                            in_=pe[:, :, 0:NK * COUT].rearrange(
                                "p c (k o) -> p k c o", k=NK))
                    # ONE 121-partition DMA writes both grids + gap zeros
                    gA = gsAB[:]
                    AS = gA.ap[0][0]
                    RUN = XG * 2 * COUT
                    dstA = AP(gws_ap.tensor,
                              (kl * TAPC + 2 * GP + xg * XG + GPAD) * 128,
                              [[GP * 128, 121], [TAPC * 128, NK],
                               [1, RUN]])
                    srcA = AP(gA.tensor, gA.offset,
                              [[AS, 121], [RUN, NK], [1, RUN]])
                    nc.sync.dma_start(out=dstA, in_=srcA)

                # ---- gather + combine for this tap group ----
                for k in (() if "gather" in debug_skip else range(kl, kh)):
                    for h in range(2):
                        gg = gatp.tile([128, HSLOT, 256], dt.bfloat16,
                                       tag="gg")
                        idxs = idx_w[:, k, h, :]
                        src = AP(gws_ap.tensor, k * TAPC * 128,
                                 [[128, TAPC], [1, 256]])
                        nc.gpsimd.dma_gather(
                            out_ap=gg[:], in_ap=src, idxs_ap=idxs,
                            num_idxs=HALF, num_idxs_reg=HALF,
                            elem_size=256, elem_step=128,
                            single_packet=False)
                        gq = gg[:].rearrange("p s (m c) -> p s m c", m=4)
                        gq2 = gg[:].rearrange(
                            "p s (m c two) -> p s m c two", m=4, two=2)
                        for m in range(4):
                            wm = wq2[:, k, m, h * HSLOT:(h + 1) * HSLOT, :]
                            wmB = wm[:, :, None, :].to_broadcast(
                                (128, HSLOT, COUT // 2, 2))
                            nc.vector.tensor_mul(out=gq2[:, :, m],
                                                 in0=gq2[:, :, m], in1=wmB)
                        nc.vector.tensor_add(out=gq[:, :, 0, :],
                                             in0=gq[:, :, 0, :],
                                             in1=gq[:, :, 1, :])
                        nc.vector.tensor_add(out=gq[:, :, 2, :],
                                             in0=gq[:, :, 2, :],
                                             in1=gq[:, :, 3, :])
                        nc.vector.tensor_add(out=gq[:, :, 0, :],
                                             in0=gq[:, :, 0, :],
                                             in1=gq[:, :, 2, :])
                        a = acc[:, h * HSLOT:(h + 1) * HSLOT, :]
                        nc.vector.tensor_add(out=a, in0=a,
                                             in1=gq[:, :, 0, :])

            gat_cm.__exit__(None, None, None)
            gsA_cm.__exit__(None, None, None)
            psO2_cm.__exit__(None, None, None)
            psO_cm.__exit__(None, None, None)
            psE_cm.__exit__(None, None, None)

            # out: [128, 98, 64] -> DRAM [12544, 64], q = qblk*128 + q_lo
            oap2 = out_t[:]
            dst = AP(oap2.tensor, 0, [[COUT, 128], [128 * COUT, NQB],
                                      [1, COUT]])
            nc.sync.dma_start(out=dst, in_=acc[:])

    nc.compile()
    return nc


def _host_prep(x):
    """Per-core input prep: pad=1 + bf16 + channel-major, plus a one-row-
    shifted copy on channels 64:128 (offset-conv tap-pair folding)."""
    import ml_dtypes
    xp = np.zeros((2 * CIN, NPAD, NPAD), np.float32)
    xp[0:CIN, 1:113, 1:113] = x
    xp[CIN:, 0:NPAD - 1, :] = xp[0:CIN, 1:NPAD, :]
    return np.ascontiguousarray(
        xp.reshape(2 * CIN, NPAD * NPAD)).astype(ml_dtypes.bfloat16)


def build(x, offset_w, offset_b, deform_w, deform_b, debug_skip=()):
    weights = {
        "offset_w": np.asarray(offset_w, np.float32),
        "offset_b": np.asarray(offset_b, np.float32),
        "deform_w": np.asarray(deform_w, np.float32),
        "deform_b": np.asarray(deform_b, np.float32),
    }
    nc = _build_program(weights, debug_skip=debug_skip)
    x = np.asarray(x, np.float32)
    in_maps = [{"xpad": _host_prep(x[b])} for b in range(x.shape[0])]
    return nc, in_maps


def _postprocess(out_maps):
    outs = []
    for om in out_maps:
        o = np.asarray(om["out_t"], np.float32)  # [HW, 64]
        outs.append(o.reshape(H, W, COUT).transpose(2, 0, 1))
    return np.stack(outs).astype(np.float32)


def kernel(x, offset_w, offset_b, deform_w, deform_b):
    from concourse import bass_utils

    nc, in_maps = build(x, offset_w, offset_b, deform_w, deform_b)
    res = bass_utils.run_bass_kernel_spmd(nc, in_maps,
                                          core_ids=list(range(len(in_maps))))
    return _postprocess(res.results)


# revision 3
# speedup vs baseline: 1.0797x; 1.0797x over previous
"""DeformableConv2d Trainium2 kernel, v2.

Data-parallel over batch: 8 samples -> 8 NeuronCores, one sample per core.

Identity: bilinear sampling commutes with the channel contraction, so
G_k = W_k @ x (1x1 conv per tap) is computed first and the bilinear gather
fetches G directly; the gathered values only need a per-pixel weighted
4-sum (no post-gather matmul).

Key improvements over v1:
  * Gather grid stores row-PAIR cells in two parity copies (A: rows
    (2y,2y+1), B: rows (2y+1,2y+2)).  One dma_gather element of 512B
    (elem_size=256, elem_step=128) fetches the full 2x2 bilinear quad
    [G(y0,x0),G(y1,x0),G(y0,x1),G(y1,x1)] regardless of y0 parity ->
    one gather per (tap, half) at full DMA bandwidth (512B descriptors
    avoid the <512B 2x DMA latency penalty).
  * Combine uses premultiplied bilinear weights (4 mul + 4 add per quad)
    with weights duplicated pairwise in memory so every DVE op has a
    packed bf16 last dim -> 2x DVE mode throughout.
  * Offset conv folds tap pairs (ki,ki+1) into K=128 matmuls using a
    row-shifted copy of x on partitions 64..127.
  * G conv runs even/odd image-row matmuls so PSUM drains produce
    pair-partitioned staging tiles; the A-grid DMA write is fully
    contiguous (4KB runs) and the B-grid tile is built by two cheap
    DVE copies.
"""

import numpy as np

K = 3
B, CIN, COUT, H, W = 8, 64, 64, 112, 112
HW = H * W                 # 12544
NPAD = H + 2               # 114
NQB = HW // 128            # 98
NPLANE = 2 * K * K         # 18
HALF = HW // 2             # 6272
HSLOT = HALF // 128        # 49
GPAD = 4
GP = H + 2 * GPAD          # 120 grid columns
# Pair-row grid layout per tap (rows of GP cells each):
#   rows 0..59   : A-parity pairs (2y, 2y+1), data in rows 2..57
#   rows 66..122 : B-parity pairs (2y+1, 2y+2) at row 65+yp', data rows
#                  66..122; rows 58..65 and 123 stay zero
# B at a 64-aligned partition offset lets one 121-partition DMA write
# both grids (partition p -> grid row p+2).
BROW0 = 65                 # B pair yp' -> grid row BROW0 + yp'
POFF = BROW0 * GP          # cell-index offset for parity 1 (7800)
NROWS = 124                # grid rows per tap
TAPC = NROWS * GP          # 14880 cells per tap
XG = 16                    # image columns per staging group
NXG = W // XG              # 7
M23 = 8388608.0

# offset-conv rounds: tap pairs (k, k+3) share kj and consecutive ki, so
# partitions 64..127 (x shifted one row down) provide tap k+3's window.
OC_ROUNDS = [(0, 3), (1, 4), (2, 5), (6,), (7,), (8,)]


def _build_program(weights, debug_skip=()):
    import ml_dtypes
    import concourse.bass as bass
    import concourse.bacc as bacc
    import concourse.mybir as mybir
    from concourse.tile import TileContext

    dt = mybir.dt
    ALU = mybir.AluOpType
    AP = bass.AP
    bf = ml_dtypes.bfloat16

    nc = bacc.Bacc("TRN2", target_bir_lowering=False, debug=False)

    offset_w = weights["offset_w"]  # [18, 64, 3, 3]
    offset_b = weights["offset_b"]  # [18]
    deform_w = weights["deform_w"]  # [64, 64, 3, 3]
    deform_b = weights["deform_b"]  # [64]

    # ---- host-side constants ----
    # offset conv lhsT per round: [128, 18]; rows 0:64 = first tap of the
    # round, rows 64:128 = second tap (zero for singleton rounds).
    woff = np.zeros((len(OC_ROUNDS), 2 * CIN, NPLANE), np.float32)
    for r, taps in enumerate(OC_ROUNDS):
        for t, k in enumerate(taps):
            ki, kj = k // 3, k % 3
            for j in range(9):
                woff[r, t * CIN:(t + 1) * CIN, j] = offset_w[2 * j, :, ki, kj]
                woff[r, t * CIN:(t + 1) * CIN, 9 + j] = \
                    offset_w[2 * j + 1, :, ki, kj]

    # G conv rhs: wg[c, k*64+o] = deform_w[o, c, ki, kj]
    wg = np.zeros((CIN, 9 * COUT), np.float32)
    for k in range(9):
        ki, kj = k // 3, k % 3
        wg[:, k * COUT:(k + 1) * COUT] = deform_w[:, :, ki, kj].T

    # base sampling grid [128, 18, 98] fp32 (+GPAD baked in, offset_b folded)
    qs = np.arange(HW)
    ho, wo = qs // W, qs % W
    base = np.zeros((NPLANE, HW), np.float32)
    for k in range(9):
        ki, kj = k // 3, k % 3
        base[k] = ho + ki - 1 + GPAD + float(offset_b[2 * k])
        base[9 + k] = wo + kj - 1 + GPAD + float(offset_b[2 * k + 1])
    base_re = np.ascontiguousarray(
        base.reshape(NPLANE, NQB, 128).transpose(2, 0, 1))  # [128, 18, 98]

    bias_rep = np.broadcast_to(
        deform_b.astype(np.float32)[None, :], (128, COUT))

    woff_c = nc.inline_tensor(
        np.ascontiguousarray(woff.transpose(1, 0, 2)).reshape(
            2 * CIN, len(OC_ROUNDS) * NPLANE).astype(bf), name="woffc")
    wg_c = nc.inline_tensor(wg.astype(bf), name="wgc")
    base_c = nc.inline_tensor(
        base_re.reshape(128, NPLANE * NQB), name="basec")
    bias_c = nc.inline_tensor(np.ascontiguousarray(bias_rep).astype(bf),
                              name="biasc")
    ident_c = nc.inline_tensor(np.eye(NPLANE, dtype=np.float32).astype(bf),
                               name="ident")
    # selection matrices for the 16-partition wrap fold (fp32: the folded
    # values are cell indices up to 14399, exact in fp32 matmuls)
    sel = np.zeros((8, 128, 16), np.float32)
    for s in range(8):
        for t in range(16):
            sel[s, 16 * s + t, t] = 1.0
    sel_c = nc.inline_tensor(
        np.ascontiguousarray(sel.transpose(1, 0, 2)).reshape(
            128, 8 * 16), name="selc")

    xpad = nc.dram_tensor("xpad", [2 * CIN, NPAD * NPAD], dt.bfloat16,
                          kind="ExternalInput")
    out_t = nc.dram_tensor("out_t", [HW, COUT], dt.bfloat16,
                           kind="ExternalOutput")
    gws = nc.dram_tensor("gws", [9 * TAPC + 8, 128], dt.bfloat16,
                         kind="Internal")
    gws_ap = gws[:, :]
    gwz = nc.dram_tensor("gwz", [76800], dt.bfloat16, kind="Internal")
    gwz_ap = gwz[:]

    with TileContext(nc) as tc:
        with (
            tc.tile_pool(name="const", bufs=1) as constp,
            tc.tile_pool(name="xsb", bufs=1) as xsbp,
            tc.tile_pool(name="persist", bufs=1) as perp,
            tc.tile_pool(name="accp", bufs=1) as accp,
            tc.tile_pool(name="zz", bufs=1) as zzp,
        ):
            # ---- constants + input ----
            woff_sb = constp.tile([2 * CIN, len(OC_ROUNDS), NPLANE],
                                  dt.bfloat16)
            nc.sync.dma_start(
                out=woff_sb[:],
                in_=woff_c[:].rearrange("c (r m) -> c r m",
                                        r=len(OC_ROUNDS)))
            wg_sb = constp.tile([CIN, 9 * COUT], dt.bfloat16)
            nc.sync.dma_start(out=wg_sb[:], in_=wg_c[:])
            base_sb = constp.tile([128, NPLANE, NQB], dt.float32)
            nc.sync.dma_start(
                out=base_sb[:],
                in_=base_c[:].rearrange("p (a b) -> p a b", a=NPLANE))
            bias_sb = constp.tile([128, COUT], dt.bfloat16)
            nc.sync.dma_start(out=bias_sb[:], in_=bias_c[:])
            ident_sb = constp.tile([NPLANE, NPLANE], dt.bfloat16)
            nc.sync.dma_start(out=ident_sb[:], in_=ident_c[:])
            sel_sb = constp.tile([128, 8, 16], dt.float32)
            nc.sync.dma_start(
                out=sel_sb[:], in_=sel_c[:].rearrange("p (s t) -> p s t", s=8))

            # x2: partitions 0:64 = padded x; 64:128 = same shifted one row
            # down (for the offset-conv tap-pair folding). Built on host.
            x2 = xsbp.tile([128, NPAD, NPAD], dt.bfloat16)
            nc.sync.dma_start(
                out=x2[:],
                in_=xpad[:].rearrange("c (a b) -> c a b", a=NPAD))
            x2ap = x2[:]
            X2S = x2ap.ap[0][0]

            zz = zzp.tile([16, 1920], dt.bfloat16)
            nc.gpsimd.memset(zz[:], 0)
            zzap = zz[:]
            ZS = zzap.ap[0][0]

            # ---- zero-fill of never-written gather-grid cells ----
            # gwz: DRAM zero pool (real-stride source for the cell fills)
            for j in range(5):
                dst = AP(gwz_ap.tensor, j * 15360,
                         [[1920, 8], [1, 1920]])
                src = AP(zzap.tensor, zzap.offset,
                         [[ZS, 8], [1, 1920]])
                nc.sync.dma_start(out=dst, in_=src)
            if "zfill" not in debug_skip:
                for k in range(9):
                    kb = k * TAPC * 128
                    # full zero rows {0,1} and {123} (cell-contiguous runs)
                    for c0, ncell in ((0, 2 * GP), (123 * GP, GP)):
                        dst = AP(gws_ap.tensor, kb + c0 * 128,
                                 [[256, ncell // 2], [1, 256]])
                        src = AP(gwz_ap.tensor, 0,
                                 [[256, ncell // 2], [1, 256]])
                        nc.sync.dma_start(out=dst, in_=src)
                    # x-border cells (4 left + 4 right) for rows 2..122
                    # (covers A data rows, the gap rows, and B data rows)
                    for xoff in (0, GP - GPAD):
                        dst = AP(gws_ap.tensor, kb + (2 * GP + xoff) * 128,
                                 [[GP * 128, 121], [1, 512]])
                        src = AP(gwz_ap.tensor, 0,
                                 [[512, 121], [1, 512]])
                        nc.sync.dma_start(out=dst, in_=src)
                # trailing pad rows (tap 8's gather AP overreads 256 els)
                dst = AP(gws_ap.tensor, 9 * TAPC * 128, [[1, 8 * 128]])
                src = AP(gwz_ap.tensor, 0, [[1, 8 * 128]])
                nc.sync.dma_start(out=dst, in_=src)

            # ---- staging pools + helper (opened before the folds so
            # PE can stage the first x-groups while DVE/ACT finish the
            # index pipeline) ----
            psE_cm = tc.tile_pool(name="psE", bufs=1, space="PSUM")
            psE = psE_cm.__enter__()
            psO_cm = tc.tile_pool(name="psO", bufs=1, space="PSUM")
            psO = psO_cm.__enter__()
            psO2_cm = tc.tile_pool(name="psO2", bufs=1, space="PSUM")
            psO2 = psO2_cm.__enter__()
            gsA_cm = tc.tile_pool(name="gsA", bufs=1)
            gsAp = gsA_cm.__enter__()
            # combined staging tiles (manual double buffer so the one-time
            # gap memset keeps a single tensor identity):
            #   partition p -> grid row p+2
            #   p 0..55   = A pairs (rows 2..57)
            #   p 56..63  = zero gap rows 58..65 (memset once)
            #   p 64..120 = B pairs (rows 66..122)
            gsAB0 = gsAp.tile([121, 5, XG, 2, COUT], dt.bfloat16,
                              tag="gsAB0")
            gsAB1 = gsAp.tile([121, 5, XG, 2, COUT], dt.bfloat16,
                              tag="gsAB1")
            nc.gpsimd.memset(gsAB0[32:64], 0)
            nc.gpsimd.memset(gsAB1[32:64], 0)

            def stage(kl, kh, xgs):
                NK = kh - kl
                for xg in xgs:
                    gsAB = (gsAB0, gsAB1)[xg % 2]
                    for cg in range(XG // 2):
                        cols = (xg * XG + cg * 2, xg * XG + cg * 2 + 1)
                        pe = psE.tile([57, 2, 512], dt.float32, tag="pe")
                        po = psO.tile([56, 2, 512], dt.float32, tag="po")
                        po2 = psO2.tile([57, 2, 512], dt.float32,
                                        tag="po2")
                        for cc, xc in enumerate(cols):
                            for ps, row0, nr in ((pe, 1, 57), (po, 2, 56),
                                                 (po2, 0, 57)):
                                lhsT = AP(x2ap.tensor,
                                          x2ap.offset + row0 * NPAD
                                          + xc + 1,
                                          [[X2S, CIN], [2 * NPAD, nr]])
                                nc.tensor.matmul(
                                    ps[0:nr, cc, 0:NK * COUT], lhsT,
                                    wg_sb[:, kl * COUT:kh * COUT],
                                    start=True, stop=True)
                        c0 = cg * 2
                        # A-h0 <- e[0:56], A-h1 <- o
                        nc.scalar.copy(
                            out=gsAB[0:56, 0:NK, c0:c0 + 2, 0, :],
                            in_=pe[0:56, :, 0:NK * COUT].rearrange(
                                "p c (k o) -> p k c o", k=NK))
                        nc.scalar.copy(
                            out=gsAB[0:56, 0:NK, c0:c0 + 2, 1, :],
                            in_=po[:, :, 0:NK * COUT].rearrange(
                                "p c (k o) -> p k c o", k=NK))
                        # B-h0 <- o2, B-h1 <- e (partition start 64)
                        nc.scalar.copy(
                            out=gsAB[64:121, 0:NK, c0:c0 + 2, 0, :],
                            in_=po2[:, :, 0:NK * COUT].rearrange(
                                "p c (k o) -> p k c o", k=NK))
                        nc.scalar.copy(
                            out=gsAB[64:121, 0:NK, c0:c0 + 2, 1, :],
                            in_=pe[:, :, 0:NK * COUT].rearrange(
                                "p c (k o) -> p k c o", k=NK))
                    # ONE 121-partition DMA writes both grids + gap zeros
                    gA = gsAB[:]
                    AS = gA.ap[0][0]
                    RUN = XG * 2 * COUT
                    dstA = AP(gws_ap.tensor,
                              (kl * TAPC + 2 * GP + xg * XG + GPAD) * 128,
                              [[GP * 128, 121], [TAPC * 128, NK],
                               [1, RUN]])
                    srcA = AP(gA.tensor, gA.offset,
                              [[AS, 121], [RUN, NK], [1, RUN]])
                    nc.sync.dma_start(out=dstA, in_=srcA)


            # ---- offset conv + index math (scoped pools) ----
            ab = tc.tile_pool(name="idxm", bufs=1)
            idxp = ab.__enter__()
            psA_cm = tc.tile_pool(name="psA", bufs=2, space="PSUM")
            psA = psA_cm.__enter__()

            off_sb = idxp.tile([NPLANE, HW], dt.bfloat16)
            RB = 4
            for blk in range(H // RB):
                ps = psA.tile([NPLANE, RB * W], dt.float32)
                for r, taps in enumerate(OC_ROUNDS):
                    ki, kj = taps[0] // 3, taps[0] % 3
                    rhs = x2[:, blk * RB + ki: blk * RB + ki + RB,
                             kj: kj + W]
                    nc.tensor.matmul(ps[:], woff_sb[:, r, :], rhs,
                                     start=(r == 0),
                                     stop=(r == len(OC_ROUNDS) - 1))
                nc.scalar.copy(
                    out=off_sb[:, blk * RB * W:(blk + 1) * RB * W], in_=ps[:])

            psA_cm.__exit__(None, None, None)
            psT_cm = tc.tile_pool(name="psT", bufs=2, space="PSUM")
            psT = psT_cm.__enter__()

            # [18, HW] -> [128, 18, 98] via PE transposes (4 per drain)
            off_re = idxp.tile([128, NPLANE, NQB], dt.float32)
            for qb0 in range(0, NQB, 4):
                nq = min(4, NQB - qb0)
                pst = psT.tile([128, 4, NPLANE], dt.bfloat16, tag="pst")
                for j in range(nq):
                    qb = qb0 + j
                    nc.tensor.transpose(
                        pst[:, j, :], off_sb[:, qb * 128:(qb + 1) * 128],
                        ident_sb[:])
                nc.scalar.copy(
                    out=off_re[:, :, qb0:qb0 + nq],
                    in_=pst[:, 0:nq, :].rearrange("p a b -> p b a"))

            psT_cm.__exit__(None, None, None)

            # ---- index math (DVE, [128, 18, 98] fp32) ----
            pyx = idxp.tile([128, NPLANE, NQB], dt.float32)
            nc.vector.tensor_add(out=pyx[:], in0=off_re[:], in1=base_sb[:])
            rnd = idxp.tile([128, NPLANE, NQB], dt.float32)
            nc.vector.tensor_scalar(rnd[:], pyx[:], M23, M23, ALU.add,
                                    ALU.subtract)
            tmp = idxp.tile([128, NPLANE, NQB], dt.float32)
            nc.vector.tensor_tensor(out=tmp[:], in0=rnd[:], in1=pyx[:],
                                    op=ALU.is_gt)
            flr = rnd
            nc.vector.tensor_sub(out=flr[:], in0=rnd[:], in1=tmp[:])
            frac = pyx
            nc.vector.tensor_sub(out=frac[:], in0=pyx[:], in1=flr[:])

            # premultiplied bilinear weights wq[m] for quad order
            # [ (y0,x0), (y1,x0), (y0,x1), (y1,x1) ]
            nc.vector.tensor_scalar(tmp[:], frac[:], -1.0, 1.0, ALU.mult,
                                    ALU.add)            # tmp = 1 - frac
            wqf = idxp.tile([128, 9, 4, NQB], dt.float32)
            nc.vector.tensor_mul(out=wqf[:, :, 0, :], in0=tmp[:, 0:9],
                                 in1=tmp[:, 9:18])
            nc.vector.tensor_mul(out=wqf[:, :, 1, :], in0=frac[:, 0:9],
                                 in1=tmp[:, 9:18])
            nc.vector.tensor_mul(out=wqf[:, :, 2, :], in0=tmp[:, 0:9],
                                 in1=frac[:, 9:18])
            nc.vector.tensor_mul(out=wqf[:, :, 3, :], in0=frac[:, 0:9],
                                 in1=frac[:, 9:18])
            # duplicate each weight twice (packed last dim -> 2x DVE mode
            # for the combine multiplies)
            wq2 = perp.tile([128, 9, 4, NQB, 2], dt.bfloat16)
            nc.gpsimd.tensor_copy(
                out=wq2[:],
                in_=wqf[:, :, :, :, None].to_broadcast(
                    (128, 9, 4, NQB, 2)))

            # cell index: yh = floor(y0/2), parity = y0 - 2*yh,
            # cell = parity*7200 + yh*120 + x0
            t1 = idxp.tile([128, 9, NQB], dt.float32, tag="t1")
            t2 = idxp.tile([128, 9, NQB], dt.float32, tag="t2")
            y0 = flr[:, 0:9]
            x0 = flr[:, 9:18]
            nc.vector.tensor_scalar_mul(t1[:], y0, 0.5)
            nc.vector.tensor_scalar(t2[:], t1[:], M23, M23, ALU.add,
                                    ALU.subtract)
            nc.vector.tensor_tensor(out=t1[:], in0=t2[:], in1=t1[:],
                                    op=ALU.is_gt)
            nc.vector.tensor_sub(out=t2[:], in0=t2[:], in1=t1[:])  # t2=yh
            nc.vector.scalar_tensor_tensor(
                out=t1[:], in0=t2[:], scalar=-2.0, in1=y0,
                op0=ALU.mult, op1=ALU.add)                       # t1=parity
            nc.vector.scalar_tensor_tensor(
                out=t2[:], in0=t2[:], scalar=float(GP), in1=x0,
                op0=ALU.mult, op1=ALU.add)                       # t2=yh*120+x0
            nc.vector.scalar_tensor_tensor(
                out=t1[:], in0=t1[:], scalar=float(POFF), in1=t2[:],
                op0=ALU.mult, op1=ALU.add)                       # t1=cell
            # +0.25 guards the fp32->int16 truncation in the drain below
            nc.vector.tensor_scalar_add(t1[:], t1[:], 0.25)

            if "gstage" not in debug_skip:
                stage(0, 5, (0, 1))

            # fold partition q_lo=(16s+t) -> wrapped [16, ...] layout via
            # fp32 selection matmuls (values <= 14879.25, exact); drain
            # converts directly to int16.
            psF_cm = tc.tile_pool(name="psF", bufs=2, space="PSUM")
            psF = psF_cm.__enter__()
            idx_w16 = idxp.tile([16, 9, 2, HSLOT, 8], dt.int16)
            for s in range(8):
                for pc in range(5):   # plane pairs (8 planes) + single
                    npl = 2 if pc < 4 else 1
                    pst2 = psF.tile([16, 2 * NQB], dt.float32, tag="pst2")
                    nc.tensor.matmul(pst2[:, 0:npl * NQB], sel_sb[:, s, :],
                                     t1[:, 2 * pc:2 * pc + npl, :],
                                     start=True, stop=True)
                    nc.scalar.copy(
                        out=idx_w16[:, 2 * pc:2 * pc + npl, :, :, s],
                        in_=pst2[:, 0:npl * NQB].rearrange(
                            "p (a h q) -> p a h q", a=npl, h=2))
            # replicate partitions 0:16 -> all 128 (3 doubling DMAs)
            idx_w = perp.tile([128, 9, 2, 8 * HSLOT], dt.int16)
            i16 = idx_w16[:]
            wap = idx_w[:]
            WSPAN = wap.ap[0][0]
            ISPAN = i16.ap[0][0]
            FREE = 9 * 2 * HSLOT * 8
            nc.sync.dma_start(
                out=AP(wap.tensor, wap.offset, [[WSPAN, 16], [1, FREE]]),
                in_=AP(i16.tensor, i16.offset, [[ISPAN, 16], [1, FREE]]))
            for np_ in (16, 32, 64):
                nc.sync.dma_start(
                    out=AP(wap.tensor, wap.offset + np_ * WSPAN,
                           [[WSPAN, np_], [1, FREE]]),
                    in_=AP(wap.tensor, wap.offset,
                           [[WSPAN, np_], [1, FREE]]))
            psF_cm.__exit__(None, None, None)
            ab.__exit__(None, None, None)

            gat_cm = tc.tile_pool(name="gat", bufs=2)
            gatp = gat_cm.__enter__()

            # ---- accumulator init ----
            acc = accp.tile([128, NQB, COUT], dt.bfloat16)
            nc.gpsimd.tensor_copy(
                out=acc[:],
                in_=bias_sb[:, None, :].to_broadcast((128, NQB, COUT)))

            def gather_combine(kl, kh):
                for k in (() if "gather" in debug_skip else range(kl, kh)):
                    for h in range(2):
                        gg = gatp.tile([128, HSLOT, 256], dt.bfloat16,
                                       tag="gg")
                        idxs = idx_w[:, k, h, :]
                        src = AP(gws_ap.tensor, k * TAPC * 128,
                                 [[128, TAPC], [1, 256]])
                        nc.gpsimd.dma_gather(
                            out_ap=gg[:], in_ap=src, idxs_ap=idxs,
                            num_idxs=HALF, num_idxs_reg=HALF,
                            elem_size=256, elem_step=128,
                            single_packet=False)
                        gq = gg[:].rearrange("p s (m c) -> p s m c", m=4)
                        gq2 = gg[:].rearrange(
                            "p s (m c two) -> p s m c two", m=4, two=2)
                        for m in range(4):
                            wm = wq2[:, k, m,
                                     h * HSLOT:(h + 1) * HSLOT, :]
                            wmB = wm[:, :, None, :].to_broadcast(
                                (128, HSLOT, COUT // 2, 2))
                            nc.vector.tensor_mul(out=gq2[:, :, m],
                                                 in0=gq2[:, :, m], in1=wmB)
                        nc.vector.tensor_add(out=gq[:, :, 0, :],
                                             in0=gq[:, :, 0, :],
                                             in1=gq[:, :, 1, :])
                        nc.vector.tensor_add(out=gq[:, :, 2, :],
                                             in0=gq[:, :, 2, :],
                                             in1=gq[:, :, 3, :])
                        nc.vector.tensor_add(out=gq[:, :, 0, :],
                                             in0=gq[:, :, 0, :],
                                             in1=gq[:, :, 2, :])
                        a = acc[:, h * HSLOT:(h + 1) * HSLOT, :]
                        nc.vector.tensor_add(out=a, in0=a,
                                             in1=gq[:, :, 0, :])

            if "gstage" not in debug_skip:
                stage(0, 5, range(2, NXG))
                gather_combine(0, 5)
                stage(5, 9, range(NXG))
                gather_combine(5, 9)

            gat_cm.__exit__(None, None, None)
            gsA_cm.__exit__(None, None, None)
            psO2_cm.__exit__(None, None, None)
            psO_cm.__exit__(None, None, None)
            psE_cm.__exit__(None, None, None)

            # out: [128, 98, 64] -> DRAM [12544, 64], q = qblk*128 + q_lo
            oap2 = out_t[:]
            dst = AP(oap2.tensor, 0, [[COUT, 128], [128 * COUT, NQB],
                                      [1, COUT]])
            nc.sync.dma_start(out=dst, in_=acc[:])

    nc.compile()
    return nc


def _host_prep(x):
    """Per-core input prep: pad=1 + bf16 + channel-major, plus a one-row-
    shifted copy on channels 64:128 (offset-conv tap-pair folding)."""
    import ml_dtypes
    xp = np.zeros((2 * CIN, NPAD, NPAD), np.float32)
    xp[0:CIN, 1:113, 1:113] = x
    xp[CIN:, 0:NPAD - 1, :] = xp[0:CIN, 1:NPAD, :]
    return np.ascontiguousarray(
        xp.reshape(2 * CIN, NPAD * NPAD)).astype(ml_dtypes.bfloat16)


def build(x, offset_w, offset_b, deform_w, deform_b, debug_skip=()):
    weights = {
        "offset_w": np.asarray(offset_w, np.float32),
        "offset_b": np.asarray(offset_b, np.float32),
        "deform_w": np.asarray(deform_w, np.float32),
        "deform_b": np.asarray(deform_b, np.float32),
    }
    nc = _build_program(weights, debug_skip=debug_skip)
    x = np.asarray(x, np.float32)
    in_maps = [{"xpad": _host_prep(x[b])} for b in range(x.shape[0])]
    return nc, in_maps


def _postprocess(out_maps):
    outs = []
    for om in out_maps:
        o = np.asarray(om["out_t"], np.float32)  # [HW, 64]
        outs.append(o.reshape(H, W, COUT).transpose(2, 0, 1))
    return np.stack(outs).astype(np.float32)


def kernel(x, offset_w, offset_b, deform_w, deform_b):
    from concourse import bass_utils

    nc, in_maps = build(x, offset_w, offset_b, deform_w, deform_b)
    res = bass_utils.run_bass_kernel_spmd(nc, in_maps,
                                          core_ids=list(range(len(in_maps))))
    return _postprocess(res.results)


# revision 4
# speedup vs baseline: 1.1407x; 1.0566x over previous
"""DeformableConv2d Trainium2 kernel, v2.

Data-parallel over batch: 8 samples -> 8 NeuronCores, one sample per core.

Identity: bilinear sampling commutes with the channel contraction, so
G_k = W_k @ x (1x1 conv per tap) is computed first and the bilinear gather
fetches G directly; the gathered values only need a per-pixel weighted
4-sum (no post-gather matmul).

Key improvements over v1:
  * Gather grid stores row-PAIR cells in two parity copies (A: rows
    (2y,2y+1), B: rows (2y+1,2y+2)).  One dma_gather element of 512B
    (elem_size=256, elem_step=128) fetches the full 2x2 bilinear quad
    [G(y0,x0),G(y1,x0),G(y0,x1),G(y1,x1)] regardless of y0 parity ->
    one gather per (tap, half) at full DMA bandwidth (512B descriptors
    avoid the <512B 2x DMA latency penalty).
  * Combine uses premultiplied bilinear weights (4 mul + 4 add per quad)
    with weights duplicated pairwise in memory so every DVE op has a
    packed bf16 last dim -> 2x DVE mode throughout.
  * Offset conv folds tap pairs (ki,ki+1) into K=128 matmuls using a
    row-shifted copy of x on partitions 64..127.
  * G conv runs even/odd image-row matmuls so PSUM drains produce
    pair-partitioned staging tiles; the A-grid DMA write is fully
    contiguous (4KB runs) and the B-grid tile is built by two cheap
    DVE copies.
"""

import numpy as np

K = 3
B, CIN, COUT, H, W = 8, 64, 64, 112, 112
HW = H * W                 # 12544
NPAD = H + 2               # 114
NQB = HW // 128            # 98
NPLANE = 2 * K * K         # 18
HALF = HW // 2             # 6272
HSLOT = HALF // 128        # 49
GPAD = 4
GP = H + 2 * GPAD          # 120 grid columns
# Pair-row grid layout per tap (rows of GP cells each):
#   rows 0..59   : A-parity pairs (2y, 2y+1), data in rows 2..57
#   rows 66..122 : B-parity pairs (2y+1, 2y+2) at row 65+yp', data rows
#                  66..122; rows 58..65 and 123 stay zero
# B at a 64-aligned partition offset lets one 121-partition DMA write
# both grids (partition p -> grid row p+2).
BROW0 = 65                 # B pair yp' -> grid row BROW0 + yp'
POFF = BROW0 * GP          # cell-index offset for parity 1 (7800)
NROWS = 124                # grid rows per tap
TAPC = NROWS * GP          # 14880 cells per tap
XG = 16                    # image columns per staging group
NXG = W // XG              # 7
M23 = 8388608.0

# offset-conv rounds: tap pairs (k, k+3) share kj and consecutive ki, so
# partitions 64..127 (x shifted one row down) provide tap k+3's window.
OC_ROUNDS = [(0, 3), (1, 4), (2, 5), (6,), (7,), (8,)]


def _build_program(weights, debug_skip=()):
    import ml_dtypes
    import concourse.bass as bass
    import concourse.bacc as bacc
    import concourse.mybir as mybir
    from concourse.tile import TileContext

    dt = mybir.dt
    ALU = mybir.AluOpType
    AP = bass.AP
    bf = ml_dtypes.bfloat16

    nc = bacc.Bacc("TRN2", target_bir_lowering=False, debug=False)

    offset_w = weights["offset_w"]  # [18, 64, 3, 3]
    offset_b = weights["offset_b"]  # [18]
    deform_w = weights["deform_w"]  # [64, 64, 3, 3]
    deform_b = weights["deform_b"]  # [64]

    # ---- host-side constants ----
    # offset conv lhsT per round: [128, 18]; rows 0:64 = first tap of the
    # round, rows 64:128 = second tap (zero for singleton rounds).
    woff = np.zeros((len(OC_ROUNDS), 2 * CIN, NPLANE), np.float32)
    for r, taps in enumerate(OC_ROUNDS):
        for t, k in enumerate(taps):
            ki, kj = k // 3, k % 3
            for j in range(9):
                woff[r, t * CIN:(t + 1) * CIN, j] = offset_w[2 * j, :, ki, kj]
                woff[r, t * CIN:(t + 1) * CIN, 9 + j] = \
                    offset_w[2 * j + 1, :, ki, kj]

    # G conv rhs: wg[c, k*64+o] = deform_w[o, c, ki, kj]
    wg = np.zeros((CIN, 9 * COUT), np.float32)
    for k in range(9):
        ki, kj = k // 3, k % 3
        wg[:, k * COUT:(k + 1) * COUT] = deform_w[:, :, ki, kj].T

    # base sampling grid [128, 18, 98] fp32 (+GPAD baked in, offset_b folded)
    qs = np.arange(HW)
    ho, wo = qs // W, qs % W
    base = np.zeros((NPLANE, HW), np.float32)
    for k in range(9):
        ki, kj = k // 3, k % 3
        base[k] = ho + ki - 1 + GPAD + float(offset_b[2 * k])
        base[9 + k] = wo + kj - 1 + GPAD + float(offset_b[2 * k + 1])
    base_re = np.ascontiguousarray(
        base.reshape(NPLANE, NQB, 128).transpose(2, 0, 1))  # [128, 18, 98]

    bias_rep = np.broadcast_to(
        deform_b.astype(np.float32)[None, :], (128, COUT))

    woff_c = nc.inline_tensor(
        np.ascontiguousarray(woff.transpose(1, 0, 2)).reshape(
            2 * CIN, len(OC_ROUNDS) * NPLANE).astype(bf), name="woffc")
    wg_c = nc.inline_tensor(wg.astype(bf), name="wgc")
    base_c = nc.inline_tensor(
        base_re.reshape(128, NPLANE * NQB), name="basec")
    bias_c = nc.inline_tensor(np.ascontiguousarray(bias_rep).astype(bf),
                              name="biasc")
    ident_c = nc.inline_tensor(np.eye(NPLANE, dtype=np.float32).astype(bf),
                               name="ident")
    # selection matrices for the 16-partition wrap fold (fp32: the folded
    # values are cell indices up to 14399, exact in fp32 matmuls)
    sel = np.zeros((8, 128, 16), np.float32)
    for s in range(8):
        for t in range(16):
            sel[s, 16 * s + t, t] = 1.0
    sel_c = nc.inline_tensor(
        np.ascontiguousarray(sel.transpose(1, 0, 2)).reshape(
            128, 8 * 16), name="selc")

    xpad = nc.dram_tensor("xpad", [2 * CIN, NPAD * NPAD], dt.bfloat16,
                          kind="ExternalInput")
    out_t = nc.dram_tensor("out_t", [HW, COUT], dt.bfloat16,
                           kind="ExternalOutput")
    gws = nc.dram_tensor("gws", [9 * TAPC + 8, 128], dt.bfloat16,
                         kind="Internal")
    gws_ap = gws[:, :]
    gwz = nc.dram_tensor("gwz", [76800], dt.bfloat16, kind="Internal")
    gwz_ap = gwz[:]

    with TileContext(nc) as tc:
        with (
            tc.tile_pool(name="const", bufs=1) as constp,
            tc.tile_pool(name="xsb", bufs=1) as xsbp,
            tc.tile_pool(name="persist", bufs=1) as perp,
            tc.tile_pool(name="accp", bufs=1) as accp,
            tc.tile_pool(name="zz", bufs=1) as zzp,
        ):
            # ---- constants + input ----
            woff_sb = constp.tile([2 * CIN, len(OC_ROUNDS), NPLANE],
                                  dt.bfloat16)
            nc.sync.dma_start(
                out=woff_sb[:],
                in_=woff_c[:].rearrange("c (r m) -> c r m",
                                        r=len(OC_ROUNDS)))
            wg_sb = constp.tile([CIN, 9 * COUT], dt.bfloat16)
            nc.sync.dma_start(out=wg_sb[:], in_=wg_c[:])
            base_sb = constp.tile([128, NPLANE, NQB], dt.float32)
            nc.sync.dma_start(
                out=base_sb[:],
                in_=base_c[:].rearrange("p (a b) -> p a b", a=NPLANE))
            bias_sb = constp.tile([128, COUT], dt.bfloat16)
            nc.sync.dma_start(out=bias_sb[:], in_=bias_c[:])
            ident_sb = constp.tile([NPLANE, NPLANE], dt.bfloat16)
            nc.sync.dma_start(out=ident_sb[:], in_=ident_c[:])
            sel_sb = constp.tile([128, 8, 16], dt.float32)
            nc.sync.dma_start(
                out=sel_sb[:], in_=sel_c[:].rearrange("p (s t) -> p s t", s=8))

            # x2: partitions 0:64 = padded x; 64:128 = same shifted one row
            # down (for the offset-conv tap-pair folding). Built on host.
            x2 = xsbp.tile([128, NPAD, NPAD], dt.bfloat16)
            nc.sync.dma_start(
                out=x2[:],
                in_=xpad[:].rearrange("c (a b) -> c a b", a=NPAD))
            x2ap = x2[:]
            X2S = x2ap.ap[0][0]

            zz = zzp.tile([16, 1920], dt.bfloat16)
            nc.gpsimd.memset(zz[:], 0)
            zzap = zz[:]
            ZS = zzap.ap[0][0]

            # ---- zero-fill of never-written gather-grid cells ----
            # gwz: DRAM zero pool (real-stride source for the cell fills)
            for j in range(5):
                dst = AP(gwz_ap.tensor, j * 15360,
                         [[1920, 8], [1, 1920]])
                src = AP(zzap.tensor, zzap.offset,
                         [[ZS, 8], [1, 1920]])
                nc.sync.dma_start(out=dst, in_=src)
            if "zfill" not in debug_skip:
                for k in range(9):
                    kb = k * TAPC * 128
                    # full zero rows {0,1} and {123} (cell-contiguous runs)
                    for c0, ncell in ((0, 2 * GP), (123 * GP, GP)):
                        dst = AP(gws_ap.tensor, kb + c0 * 128,
                                 [[256, ncell // 2], [1, 256]])
                        src = AP(gwz_ap.tensor, 0,
                                 [[256, ncell // 2], [1, 256]])
                        nc.sync.dma_start(out=dst, in_=src)
                    # x-border cells (4 left + 4 right) for rows 2..122
                    # (covers A data rows, the gap rows, and B data rows)
                    for xoff in (0, GP - GPAD):
                        dst = AP(gws_ap.tensor, kb + (2 * GP + xoff) * 128,
                                 [[GP * 128, 121], [1, 512]])
                        src = AP(gwz_ap.tensor, 0,
                                 [[512, 121], [1, 512]])
                        nc.sync.dma_start(out=dst, in_=src)
                # trailing pad rows (tap 8's gather AP overreads 256 els)
                dst = AP(gws_ap.tensor, 9 * TAPC * 128, [[1, 8 * 128]])
                src = AP(gwz_ap.tensor, 0, [[1, 8 * 128]])
                nc.sync.dma_start(out=dst, in_=src)

            # ---- staging pools + helper (opened before the folds so
            # PE can stage the first x-groups while DVE/ACT finish the
            # index pipeline) ----
            psE_cm = tc.tile_pool(name="psE", bufs=1, space="PSUM")
            psE = psE_cm.__enter__()
            psO_cm = tc.tile_pool(name="psO", bufs=1, space="PSUM")
            psO = psO_cm.__enter__()
            psO2_cm = tc.tile_pool(name="psO2", bufs=1, space="PSUM")
            psO2 = psO2_cm.__enter__()
            gsA_cm = tc.tile_pool(name="gsA", bufs=1)
            gsAp = gsA_cm.__enter__()
            # combined staging tiles (manual double buffer so the one-time
            # gap memset keeps a single tensor identity):
            #   partition p -> grid row p+2
            #   p 0..55   = A pairs (rows 2..57)
            #   p 56..63  = zero gap rows 58..65 (memset once)
            #   p 64..120 = B pairs (rows 66..122)
            gsAB0 = gsAp.tile([121, 5, XG, 2, COUT], dt.bfloat16,
                              tag="gsAB0")
            gsAB1 = gsAp.tile([121, 5, XG, 2, COUT], dt.bfloat16,
                              tag="gsAB1")
            nc.gpsimd.memset(gsAB0[32:64], 0)
            nc.gpsimd.memset(gsAB1[32:64], 0)

            def stage(kl, kh, xgs, b_on_dve=False):
                NK = kh - kl
                for xg in xgs:
                    gsAB = (gsAB0, gsAB1)[xg % 2]
                    for cg in range(XG // 2):
                        cols = (xg * XG + cg * 2, xg * XG + cg * 2 + 1)
                        pe = psE.tile([57, 2, 512], dt.float32, tag="pe")
                        po = psO.tile([56, 2, 512], dt.float32, tag="po")
                        po2 = psO2.tile([57, 2, 512], dt.float32,
                                        tag="po2")
                        for cc, xc in enumerate(cols):
                            for ps, row0, nr in ((pe, 1, 57), (po, 2, 56),
                                                 (po2, 0, 57)):
                                lhsT = AP(x2ap.tensor,
                                          x2ap.offset + row0 * NPAD
                                          + xc + 1,
                                          [[X2S, CIN], [2 * NPAD, nr]])
                                nc.tensor.matmul(
                                    ps[0:nr, cc, 0:NK * COUT], lhsT,
                                    wg_sb[:, kl * COUT:kh * COUT],
                                    start=True, stop=True)
                        c0 = cg * 2
                        # A-h0 <- e[0:56], A-h1 <- o
                        nc.scalar.copy(
                            out=gsAB[0:56, 0:NK, c0:c0 + 2, 0, :],
                            in_=pe[0:56, :, 0:NK * COUT].rearrange(
                                "p c (k o) -> p k c o", k=NK))
                        nc.scalar.copy(
                            out=gsAB[0:56, 0:NK, c0:c0 + 2, 1, :],
                            in_=po[:, :, 0:NK * COUT].rearrange(
                                "p c (k o) -> p k c o", k=NK))
                        # B-h0 <- o2, B-h1 <- e (partition start 64);
                        # group 0 drains B on the phase-1-idle DVE
                        bcp = (nc.vector.tensor_copy if b_on_dve
                               else nc.scalar.copy)
                        bcp(out=gsAB[64:121, 0:NK, c0:c0 + 2, 0, :],
                            in_=po2[:, :, 0:NK * COUT].rearrange(
                                "p c (k o) -> p k c o", k=NK))
                        bcp(out=gsAB[64:121, 0:NK, c0:c0 + 2, 1, :],
                            in_=pe[:, :, 0:NK * COUT].rearrange(
                                "p c (k o) -> p k c o", k=NK))
                    # ONE 121-partition DMA writes both grids + gap zeros
                    gA = gsAB[:]
                    AS = gA.ap[0][0]
                    RUN = XG * 2 * COUT
                    dstA = AP(gws_ap.tensor,
                              (kl * TAPC + 2 * GP + xg * XG + GPAD) * 128,
                              [[GP * 128, 121], [TAPC * 128, NK],
                               [1, RUN]])
                    srcA = AP(gA.tensor, gA.offset,
                              [[AS, 121], [RUN, NK], [1, RUN]])
                    nc.sync.dma_start(out=dstA, in_=srcA)


            # ---- offset conv + index math (scoped pools) ----
            ab = tc.tile_pool(name="idxm", bufs=1)
            idxp = ab.__enter__()
            psA_cm = tc.tile_pool(name="psA", bufs=2, space="PSUM")
            psA = psA_cm.__enter__()

            off_sb = idxp.tile([NPLANE, HW], dt.bfloat16)
            RB = 4
            for blk in range(H // RB):
                ps = psA.tile([NPLANE, RB * W], dt.float32)
                for r, taps in enumerate(OC_ROUNDS):
                    ki, kj = taps[0] // 3, taps[0] % 3
                    rhs = x2[:, blk * RB + ki: blk * RB + ki + RB,
                             kj: kj + W]
                    nc.tensor.matmul(ps[:], woff_sb[:, r, :], rhs,
                                     start=(r == 0),
                                     stop=(r == len(OC_ROUNDS) - 1))
                nc.scalar.copy(
                    out=off_sb[:, blk * RB * W:(blk + 1) * RB * W], in_=ps[:])

            psA_cm.__exit__(None, None, None)
            psT_cm = tc.tile_pool(name="psT", bufs=2, space="PSUM")
            psT = psT_cm.__enter__()

            # [18, HW] -> [128, 18, 98] via PE transposes (4 per drain)
            off_re = idxp.tile([128, NPLANE, NQB], dt.float32)
            for qb0 in range(0, NQB, 4):
                nq = min(4, NQB - qb0)
                pst = psT.tile([128, 4, NPLANE], dt.bfloat16, tag="pst")
                for j in range(nq):
                    qb = qb0 + j
                    nc.tensor.transpose(
                        pst[:, j, :], off_sb[:, qb * 128:(qb + 1) * 128],
                        ident_sb[:])
                nc.scalar.copy(
                    out=off_re[:, :, qb0:qb0 + nq],
                    in_=pst[:, 0:nq, :].rearrange("p a b -> p b a"))

            psT_cm.__exit__(None, None, None)

            # ---- index math (DVE, [128, 18, 98] fp32) ----
            pyx = idxp.tile([128, NPLANE, NQB], dt.float32)
            nc.vector.tensor_add(out=pyx[:], in0=off_re[:], in1=base_sb[:])
            rnd = idxp.tile([128, NPLANE, NQB], dt.float32)
            nc.vector.tensor_scalar(rnd[:], pyx[:], M23, M23, ALU.add,
                                    ALU.subtract)
            tmp = idxp.tile([128, NPLANE, NQB], dt.float32)
            nc.vector.tensor_tensor(out=tmp[:], in0=rnd[:], in1=pyx[:],
                                    op=ALU.is_gt)
            flr = rnd
            nc.vector.tensor_sub(out=flr[:], in0=rnd[:], in1=tmp[:])
            frac = pyx
            nc.vector.tensor_sub(out=frac[:], in0=pyx[:], in1=flr[:])

            # premultiplied bilinear weights wq[m] for quad order
            # [ (y0,x0), (y1,x0), (y0,x1), (y1,x1) ]
            nc.vector.tensor_scalar(tmp[:], frac[:], -1.0, 1.0, ALU.mult,
                                    ALU.add)            # tmp = 1 - frac
            wqf = idxp.tile([128, 9, 4, NQB], dt.float32)
            nc.vector.tensor_mul(out=wqf[:, :, 0, :], in0=tmp[:, 0:9],
                                 in1=tmp[:, 9:18])
            nc.vector.tensor_mul(out=wqf[:, :, 1, :], in0=frac[:, 0:9],
                                 in1=tmp[:, 9:18])
            nc.vector.tensor_mul(out=wqf[:, :, 2, :], in0=tmp[:, 0:9],
                                 in1=frac[:, 9:18])
            nc.vector.tensor_mul(out=wqf[:, :, 3, :], in0=frac[:, 0:9],
                                 in1=frac[:, 9:18])
            # duplicate each weight twice (packed last dim -> 2x DVE mode
            # for the combine multiplies)
            wq2 = perp.tile([128, 9, 4, NQB, 2], dt.bfloat16)
            nc.gpsimd.tensor_copy(
                out=wq2[:],
                in_=wqf[:, :, :, :, None].to_broadcast(
                    (128, 9, 4, NQB, 2)))

            # cell index: yh = floor(y0/2), parity = y0 - 2*yh,
            # cell = parity*7200 + yh*120 + x0
            t1 = idxp.tile([128, 9, NQB], dt.float32, tag="t1")
            t2 = idxp.tile([128, 9, NQB], dt.float32, tag="t2")
            y0 = flr[:, 0:9]
            x0 = flr[:, 9:18]
            nc.vector.tensor_scalar_mul(t1[:], y0, 0.5)
            nc.vector.tensor_scalar(t2[:], t1[:], M23, M23, ALU.add,
                                    ALU.subtract)
            nc.vector.tensor_tensor(out=t1[:], in0=t2[:], in1=t1[:],
                                    op=ALU.is_gt)
            nc.vector.tensor_sub(out=t2[:], in0=t2[:], in1=t1[:])  # t2=yh
            nc.vector.scalar_tensor_tensor(
                out=t1[:], in0=t2[:], scalar=-2.0, in1=y0,
                op0=ALU.mult, op1=ALU.add)                       # t1=parity
            nc.vector.scalar_tensor_tensor(
                out=t2[:], in0=t2[:], scalar=float(GP), in1=x0,
                op0=ALU.mult, op1=ALU.add)                       # t2=yh*120+x0
            nc.vector.scalar_tensor_tensor(
                out=t1[:], in0=t1[:], scalar=float(POFF), in1=t2[:],
                op0=ALU.mult, op1=ALU.add)                       # t1=cell
            # +0.25 guards the fp32->int16 truncation in the drain below
            nc.vector.tensor_scalar_add(t1[:], t1[:], 0.25)

            if "gstage" not in debug_skip:
                stage(0, 5, (0, 1), b_on_dve=True)

            # fold partition q_lo=(16s+t) -> wrapped [16, ...] layout via
            # fp32 selection matmuls (values <= 14879.25, exact); drain
            # converts directly to int16.
            psF_cm = tc.tile_pool(name="psF", bufs=2, space="PSUM")
            psF = psF_cm.__enter__()
            idx_w16 = idxp.tile([16, 9, 2, HSLOT, 8], dt.int16)
            for s in range(8):
                for pc in range(5):   # plane pairs (8 planes) + single
                    npl = 2 if pc < 4 else 1
                    pst2 = psF.tile([16, 2 * NQB], dt.float32, tag="pst2")
                    nc.tensor.matmul(pst2[:, 0:npl * NQB], sel_sb[:, s, :],
                                     t1[:, 2 * pc:2 * pc + npl, :],
                                     start=True, stop=True)
                    nc.vector.tensor_copy(
                        out=idx_w16[:, 2 * pc:2 * pc + npl, :, :, s],
                        in_=pst2[:, 0:npl * NQB].rearrange(
                            "p (a h q) -> p a h q", a=npl, h=2))
            # replicate partitions 0:16 -> all 128 (3 doubling DMAs)
            idx_w = perp.tile([128, 9, 2, 8 * HSLOT], dt.int16)
            i16 = idx_w16[:]
            wap = idx_w[:]
            WSPAN = wap.ap[0][0]
            ISPAN = i16.ap[0][0]
            FREE = 9 * 2 * HSLOT * 8
            nc.sync.dma_start(
                out=AP(wap.tensor, wap.offset, [[WSPAN, 16], [1, FREE]]),
                in_=AP(i16.tensor, i16.offset, [[ISPAN, 16], [1, FREE]]))
            for np_ in (16, 32, 64):
                nc.sync.dma_start(
                    out=AP(wap.tensor, wap.offset + np_ * WSPAN,
                           [[WSPAN, np_], [1, FREE]]),
                    in_=AP(wap.tensor, wap.offset,
                           [[WSPAN, np_], [1, FREE]]))
            psF_cm.__exit__(None, None, None)
            ab.__exit__(None, None, None)

            gat_cm = tc.tile_pool(name="gat", bufs=2)
            gatp = gat_cm.__enter__()

            # ---- accumulator init ----
            acc = accp.tile([128, NQB, COUT], dt.bfloat16)
            nc.gpsimd.tensor_copy(
                out=acc[:],
                in_=bias_sb[:, None, :].to_broadcast((128, NQB, COUT)))

            def gather_combine(kl, kh):
                for k in (() if "gather" in debug_skip else range(kl, kh)):
                    for h in range(2):
                        gg = gatp.tile([128, HSLOT, 256], dt.bfloat16,
                                       tag="gg")
                        idxs = idx_w[:, k, h, :]
                        src = AP(gws_ap.tensor, k * TAPC * 128,
                                 [[128, TAPC], [1, 256]])
                        nc.gpsimd.dma_gather(
                            out_ap=gg[:], in_ap=src, idxs_ap=idxs,
                            num_idxs=HALF, num_idxs_reg=HALF,
                            elem_size=256, elem_step=128,
                            single_packet=False)
                        gq = gg[:].rearrange("p s (m c) -> p s m c", m=4)
                        gq2 = gg[:].rearrange(
                            "p s (m c two) -> p s m c two", m=4, two=2)
                        for m in range(4):
                            wm = wq2[:, k, m,
                                     h * HSLOT:(h + 1) * HSLOT, :]
                            wmB = wm[:, :, None, :].to_broadcast(
                                (128, HSLOT, COUT // 2, 2))
                            nc.vector.tensor_mul(out=gq2[:, :, m],
                                                 in0=gq2[:, :, m], in1=wmB)
                        nc.vector.tensor_add(out=gq[:, :, 0, :],
                                             in0=gq[:, :, 0, :],
                                             in1=gq[:, :, 1, :])
                        nc.vector.tensor_add(out=gq[:, :, 2, :],
                                             in0=gq[:, :, 2, :],
                                             in1=gq[:, :, 3, :])
                        nc.vector.tensor_add(out=gq[:, :, 0, :],
                                             in0=gq[:, :, 0, :],
                                             in1=gq[:, :, 2, :])
                        a = acc[:, h * HSLOT:(h + 1) * HSLOT, :]
                        nc.vector.tensor_add(out=a, in0=a,
                                             in1=gq[:, :, 0, :])

            if "gstage" not in debug_skip:
                stage(0, 5, range(2, NXG), b_on_dve=True)
                gather_combine(0, 5)
                stage(5, 9, range(NXG))
                gather_combine(5, 9)

            gat_cm.__exit__(None, None, None)
            gsA_cm.__exit__(None, None, None)
            psO2_cm.__exit__(None, None, None)
            psO_cm.__exit__(None, None, None)
            psE_cm.__exit__(None, None, None)

            # out: [128, 98, 64] -> DRAM [12544, 64], q = qblk*128 + q_lo
            oap2 = out_t[:]
            dst = AP(oap2.tensor, 0, [[COUT, 128], [128 * COUT, NQB],
                                      [1, COUT]])
            nc.sync.dma_start(out=dst, in_=acc[:])

    nc.compile()
    return nc


def _host_prep(x):
    """Per-core input prep: pad=1 + bf16 + channel-major, plus a one-row-
    shifted copy on channels 64:128 (offset-conv tap-pair folding)."""
    import ml_dtypes
    xp = np.zeros((2 * CIN, NPAD, NPAD), np.float32)
    xp[0:CIN, 1:113, 1:113] = x
    xp[CIN:, 0:NPAD - 1, :] = xp[0:CIN, 1:NPAD, :]
    return np.ascontiguousarray(
        xp.reshape(2 * CIN, NPAD * NPAD)).astype(ml_dtypes.bfloat16)


def build(x, offset_w, offset_b, deform_w, deform_b, debug_skip=()):
    weights = {
        "offset_w": np.asarray(offset_w, np.float32),
        "offset_b": np.asarray(offset_b, np.float32),
        "deform_w": np.asarray(deform_w, np.float32),
        "deform_b": np.asarray(deform_b, np.float32),
    }
    nc = _build_program(weights, debug_skip=debug_skip)
    x = np.asarray(x, np.float32)
    in_maps = [{"xpad": _host_prep(x[b])} for b in range(x.shape[0])]
    return nc, in_maps


def _postprocess(out_maps):
    outs = []
    for om in out_maps:
        o = np.asarray(om["out_t"], np.float32)  # [HW, 64]
        outs.append(o.reshape(H, W, COUT).transpose(2, 0, 1))
    return np.stack(outs).astype(np.float32)


def kernel(x, offset_w, offset_b, deform_w, deform_b):
    from concourse import bass_utils

    nc, in_maps = build(x, offset_w, offset_b, deform_w, deform_b)
    res = bass_utils.run_bass_kernel_spmd(nc, in_maps,
                                          core_ids=list(range(len(in_maps))))
    return _postprocess(res.results)
